# revision 22
# baseline (speedup 1.0000x reference)
"""DiT (4-layer, adaLN-modulated transformer) forward on 8 TRN2 NeuronCores.

Sharding: core c -> (batch b=c//2, sequence half c%2), 512 tokens/core.
Activations are feature-major [features, tokens] on chip. All matmuls bf16
with f32 PSUM; residual stays f32 with deferred SKIP scaling (alpha folding).

v1 rewrite vs baseline:
- fat weight DMAs: one load per (layer, group) with 8KB/partition contiguous
  rows (DMA descriptor count /8); uniform [128, 4096] streaming tiles.
- adaln: fat wad loads (2/layer); mod/park vectors for ALL layers hoisted
  right after the AllToAll (off every layer's critical path).
- LayerNorm rstd via DVE bit-trick rsqrt (int shift seed + 1 Newton step):
  no ACT table loads (Ln/Exp) on the critical path at all.
- attention order k->AGk, v->AGv, q, so both AllGathers hide under compute;
  gathered k/v staging emitted after the q weight loads; per-j
  score->exp->out interleave keeps PE and ACT in lockstep.
- denominators: v lhsT carries 64 ones columns so the softmax denom lands
  replicated on PSUM partitions 64:128; reciprocal+mul on DVE, no gpsimd
  partition_broadcast.
- PSUM group psums alternate between two bank sets so back-to-back matmul
  groups double-buffer.
"""
import sys
import numpy as np

sys.path.insert(0, "/opt/trn_rl_repo")

import ml_dtypes  # noqa: E402
import concourse.bass as bass  # noqa: E402
import concourse.bacc as bacc  # noqa: E402
import concourse.tile as tile  # noqa: E402
from concourse import mybir  # noqa: E402
from concourse.bass_utils import run_bass_kernel_spmd  # noqa: E402

F32 = mybir.dt.float32
I32 = mybir.dt.int32
BF16 = mybir.dt.bfloat16
AF = mybir.ActivationFunctionType
ALU = mybir.AluOpType
BF = ml_dtypes.bfloat16

D = 1024
NL = 4
H = 16
HD = 64
B = 4
L = 1024
SCALE = HD ** (-0.5)
SKIP = 2.0 ** (-0.5)
EPS = 1e-6

NC = 8          # cores
T = 512         # tokens per core
FT = 8          # feature tiles per 1024 features
KT = 8          # k-tiles of contraction dim D
ADC = 768       # adaln column slice per core (6 tiles of 128)

PAIRS = [[0, 1], [2, 3], [4, 5], [6, 7]]
WORLD = [list(range(NC))]

RSQRT_MAGIC = 0x5F3759DF


def _deinterleave_perm():
    p = []
    for h in range(H):
        base = h * HD
        p.extend(base + np.arange(0, HD, 2))
        p.extend(base + np.arange(1, HD, 2))
    return np.array(p, np.int64)


def build(nc, use_vb, use_pb, use_m2b, use_adb):
    x_in = nc.dram_tensor("xt", [128, FT, T], F32, kind="ExternalInput")
    silu_cc = nc.dram_tensor("silu_cc", [128, KT, B], BF16, kind="ExternalInput")
    ropeC = nc.dram_tensor("ropeC", [128, T], BF16, kind="ExternalInput")
    ropeS = nc.dram_tensor("ropeS", [128, T], BF16, kind="ExternalInput")
    # fat weight layouts: one [128, 4096] (or [128, 3072] for wad) per load
    wqk = nc.dram_tensor("wqk", [NL, 4, 128, KT * 512], BF16, kind="ExternalInput")
    wv = nc.dram_tensor("wv", [NL, 2, 128, KT * 512], BF16, kind="ExternalInput")
    wproj = nc.dram_tensor("wproj", [NL, 2, 128, KT * 512], BF16,
                           kind="ExternalInput")
    wm1 = nc.dram_tensor("wm1", [NL, 8, 128, KT * 512], BF16, kind="ExternalInput")
    wm2 = nc.dram_tensor("wm2", [NL, 8, 128, KT * 512], BF16, kind="ExternalInput")
    wad = nc.dram_tensor("wad", [NL, 2, 128, 4 * ADC], BF16, kind="ExternalInput")
    bqk = nc.dram_tensor("bqk", [NL, 128, 16], F32, kind="ExternalInput")
    bm1 = nc.dram_tensor("bm1", [NL, 128, 32], F32, kind="ExternalInput")
    vb_b = bpj = bm2 = bad = None
    if use_vb:
        vb_b = nc.dram_tensor("vb_b", [NL, 128, 1024], F32, kind="ExternalInput")
    if use_pb:
        bpj = nc.dram_tensor("bpj", [NL, 128, FT], F32, kind="ExternalInput")
    if use_m2b:
        bm2 = nc.dram_tensor("bm2", [NL, 128, FT], F32, kind="ExternalInput")
    if use_adb:
        bad = nc.dram_tensor("bad", [NL, 128, 48], F32, kind="ExternalInput")
    out = nc.dram_tensor("out", [128, FT, T], F32, kind="ExternalOutput")
    import os
    _dbg = os.environ.get("KDEBUG") == "1"
    taps = {}
    if _dbg:
        for nm, shape, dt in [
            ("tap_mod", [128, 48], F32), ("tap_park", [128, 6, FT], F32),
            ("tap_h", [128, FT, T], BF16), ("tap_qk", [128, 16, T], BF16),
            ("tap_kfull", [128, 8, 1024], BF16),
            ("tap_vsb", [128, 8, H, 128], BF16),
            ("tap_o", [128, FT, T], BF16), ("tap_x1", [128, FT, T], F32),
            ("tap_rb", [128, T], F32), ("tap_mbrb", [128, T], F32),
        ]:
            taps[nm] = nc.dram_tensor(nm, shape, dt, kind="ExternalOutput")

    import contextlib
    with tile.TileContext(nc) as tc, contextlib.ExitStack() as ctx:
        # ------------- pools -------------
        singles = ctx.enter_context(tc.tile_pool(name="singles", bufs=1))
        xpool = ctx.enter_context(tc.tile_pool(name="xpool", bufs=1))
        actp = ctx.enter_context(tc.tile_pool(name="actp", bufs=1))
        kvp = ctx.enter_context(tc.tile_pool(name="kvp", bufs=1))
        wpool = ctx.enter_context(tc.tile_pool(name="wpool", bufs=3))
        ppool = ctx.enter_context(tc.tile_pool(name="ppool", bufs=3))
        small = ctx.enter_context(tc.tile_pool(name="small", bufs=2))
        scr = ctx.enter_context(tc.tile_pool(name="scr", bufs=2))
        psA = ctx.enter_context(tc.tile_pool(name="psA", bufs=4, space="PSUM"))
        psB = ctx.enter_context(tc.tile_pool(name="psB", bufs=2, space="PSUM"))
        dram = ctx.enter_context(tc.tile_pool(name="dram", bufs=1, space="DRAM"))

        # group-psum allocator: alternate psA tiles and psB halves so
        # consecutive matmul groups double-buffer across the 8 banks.
        _gp = {"n": 0}

        def group_psums(tag):
            if _gp["n"] % 2 == 0:
                ps = [psA.tile([128, 512], F32, name=f"ga_{tag}_{i}", tag="acc")
                      for i in range(4)]
            else:
                t0 = psB.tile([128, 1024], F32, name=f"gb_{tag}_0", tag="sc2")
                t1 = psB.tile([128, 1024], F32, name=f"gb_{tag}_1", tag="sc2")
                ps = [t0[:, 0:512], t0[:, 512:1024], t1[:, 0:512], t1[:, 512:1024]]
            _gp["n"] += 1
            return ps

        # ------------- persistent SBUF -------------
        x_sb = xpool.tile([128, FT, T], F32, name="x_sb")
        nc.sync.dma_start(out=x_sb[:], in_=x_in[:])

        cC = singles.tile([128, T], BF16, name="cC")
        cS = singles.tile([128, T], BF16, name="cS")
        nc.sync.dma_start(out=cC[:], in_=ropeC[:])
        nc.sync.dma_start(out=cS[:], in_=ropeS[:])

        scc = singles.tile([128, KT, B], BF16, name="scc")
        nc.sync.dma_start(out=scc[:], in_=silu_cc[:])

        bqk_sb = singles.tile([128, NL, 16], F32, name="bqk_sb")
        nc.sync.dma_start(out=bqk_sb[:], in_=bqk.ap().rearrange("l p f -> p l f"))
        bm1_sb = singles.tile([128, NL, 32], F32, name="bm1_sb")
        nc.sync.dma_start(out=bm1_sb[:], in_=bm1.ap().rearrange("l p f -> p l f"))
        vb_sb = bpj_sb = bm2_sb = bad_sb = None
        if use_vb:
            vb_sb = singles.tile([128, NL, 1024], F32, name="vb_sb")
            nc.sync.dma_start(out=vb_sb[:], in_=vb_b.ap().rearrange("l p f -> p l f"))
        if use_pb:
            bpj_sb = singles.tile([128, NL, FT], F32, name="bpj_sb")
            nc.sync.dma_start(out=bpj_sb[:], in_=bpj.ap().rearrange("l p f -> p l f"))
        if use_m2b:
            bm2_sb = singles.tile([128, NL, FT], F32, name="bm2_sb")
            nc.sync.dma_start(out=bm2_sb[:], in_=bm2.ap().rearrange("l p f -> p l f"))
        if use_adb:
            bad_sb = singles.tile([128, NL, 48], F32, name="bad_sb")
            nc.sync.dma_start(out=bad_sb[:], in_=bad.ap().rearrange("l p f -> p l f"))

        # v_sb: [tok-part, j(global 128-tok chunk), head, 64 feats + 64 ones]
        v_sb = singles.tile([128, 8, H, 128], BF16, name="v_sb")
        nc.vector.memset(v_sb[:, :, :, 64:128], 1.0)

        # =================================================================
        # adaln, column-sharded: mod[l] = silu(cc) @ adaln_w[l][:, my cols]
        # fat wad loads (2/layer); psum [128, 4] per jt chunk.
        # =================================================================
        ad_send = dram.tile([NC, NL, 128, 6], F32, name="ad_send")
        ad_gath = dram.tile([NC, NL, 128, 6], F32, name="ad_gath")

        for l in range(NL):
            mod_out = small.tile([128, B, 6], F32, name=f"mod_out_{l}",
                                 tag="mod_out")
            mpa = [psA.tile([128, 512], F32, name=f"madp_{l}_{i}", tag="acc")
                   for i in range(4)]
            mpb = psB.tile([128, 1024], F32, name=f"madpb_{l}", tag="sc2")
            mps = [mpa[0][:, 0:B], mpa[1][:, 0:B], mpa[2][:, 0:B],
                   mpa[3][:, 0:B], mpb[:, 0:B], mpb[:, 512:512 + B]]
            for c in range(2):
                wt = wpool.tile([128, 4096], BF16, name="wad_t", tag="w")
                nc.sync.dma_start(out=wt[:, 0:4 * ADC], in_=wad[l, c])
                for kk in range(4):
                    k = c * 4 + kk
                    for jt in range(6):
                        nc.tensor.matmul(
                            mps[jt],
                            lhsT=wt[:, kk * ADC + jt * 128:
                                    kk * ADC + (jt + 1) * 128],
                            rhs=scc[:, k, :],
                            start=(k == 0), stop=(k == KT - 1))
            for jt in range(6):
                nc.vector.tensor_copy(out=mod_out[:, :, jt], in_=mps[jt])
            # send[2b+e, l, p, jt] = mod_out[p, jt, b]
            for e in range(2):
                src = bass.AP(
                    tensor=mod_out.tensor, offset=mod_out.offset,
                    ap=[list(mod_out.ap[0]), [6, B], [1, 6]])
                dst = bass.AP(
                    tensor=ad_send.tensor,
                    offset=ad_send.offset + l * 128 * 6 + e * NL * 128 * 6,
                    ap=[[6, 128], [2 * NL * 128 * 6, B], [1, 6]])
                nc.gpsimd.dma_start(out=dst, in_=src)

        nc.gpsimd.collective_compute(
            "AllToAll", ALU.bypass,
            ins=[ad_send.opt()], outs=[ad_gath.opt()],
            replica_groups=WORLD)

        # ---- mod + park for ALL layers (hoisted) ----
        alpha_l = [SKIP ** (2 * i) for i in range(NL)]
        parks = []
        for l in range(NL):
            mod_sb = small.tile([128, 48], F32, name=f"mod_sb_{l}",
                                tag=f"mod_sb_{l}", bufs=1)
            src = bass.AP(
                tensor=ad_gath.tensor, offset=ad_gath.offset + l * 128 * 6,
                ap=[[6, 128], [NL * 128 * 6, NC], [1, 6]])
            nc.sync.dma_start(out=mod_sb[:], in_=src)
            if use_adb:
                nc.vector.tensor_add(mod_sb[:], mod_sb[:], bad_sb[:, l, :])
            park = small.tile([128, 6, FT], F32, name=f"park_{l}",
                              tag=f"park_{l}", bufs=1)
            a_msa = alpha_l[l]
            a_mlp = alpha_l[l] * SKIP
            nc.vector.tensor_scalar_mul(park[:, 0, :], mod_sb[:, 0:8], SKIP)
            nc.vector.tensor_scalar(park[:, 1, :], mod_sb[:, 8:16], 1.0, SKIP,
                                    ALU.add, ALU.mult)
            nc.vector.tensor_scalar_mul(park[:, 2, :], mod_sb[:, 16:24],
                                        1.0 / a_msa)
            nc.vector.tensor_scalar_mul(park[:, 3, :], mod_sb[:, 24:32], SKIP)
            nc.vector.tensor_scalar(park[:, 4, :], mod_sb[:, 32:40], 1.0, SKIP,
                                    ALU.add, ALU.mult)
            nc.vector.tensor_scalar_mul(park[:, 5, :], mod_sb[:, 40:48],
                                        1.0 / a_mlp)
            if _dbg and l == 0:
                nc.sync.dma_start(out=taps["tap_mod"].ap(), in_=mod_sb[:])
            parks.append(park)

        # =================================================================
        k_send = dram.tile([8, 128, 512], BF16, name="k_send")
        k_gath = dram.tile([2, 8, 128, 512], BF16, name="k_gath")
        v_send = dram.tile([4, 128, 1024], BF16, name="v_send")
        v_gath = dram.tile([2, 4, 128, 1024], BF16, name="v_gath")

        def ln_stats(lname):
            """casts + sum / sum-of-squares partition reductions.
            per-ft bf16 tiles rotate through scratch (SBUF thrift);
            the two accumulation chains interleave."""
            ps_s = psA.tile([128, 512], F32, name=f"ps_s_{lname}", tag="acc")
            ps_q = psA.tile([128, 512], F32, name=f"ps_q_{lname}", tag="acc")
            for ft in range(FT):
                x16 = scr.tile([128, T], BF16, name=f"x16_{lname}_{ft}",
                               tag="x16s")
                xsq = scr.tile([128, T], BF16, name=f"xsq_{lname}_{ft}",
                               tag="xsqs")
                nc.scalar.activation(out=x16[:], in_=x_sb[:, ft, :],
                                     func=AF.Copy)
                nc.vector.tensor_mul(xsq[:], x_sb[:, ft, :], x_sb[:, ft, :])
                nc.tensor.matmul(ps_s[:], lhsT=ones128[:], rhs=x16[:],
                                 start=(ft == 0), stop=(ft == FT - 1))
                nc.tensor.matmul(ps_q[:], lhsT=ones128[:], rhs=xsq[:],
                                 start=(ft == 0), stop=(ft == FT - 1))
            return ps_s, ps_q

        def ln_finish(lname, ps_s, ps_q, sc_ap, sh_ap):
            """h = (LN(x)*(1+sc)+sh)*SKIP in bf16.
            rstd via DVE bit-trick rsqrt: V = D*q - s^2 (= D^2 var),
            y ~= V^-1/2 (seed + 1 NR), rb = D*y, mbrb = s*y."""
            s_sb = small.tile([128, T], F32, name=f"ssb_{lname}", tag="lnss",
                              bufs=1)
            sq = small.tile([128, T], F32, name=f"sq_{lname}", tag="lnsq", bufs=1)
            vv = small.tile([128, T], F32, name=f"vv_{lname}", tag="lnvv", bufs=1)
            y0 = small.tile([128, T], F32, name=f"y0_{lname}", tag="lny0", bufs=1)
            t1 = small.tile([128, T], F32, name=f"t1_{lname}", tag="lnt1", bufs=1)
            nc.vector.tensor_copy(out=s_sb[:], in_=ps_s[:])
            nc.vector.tensor_mul(sq[:], s_sb[:], s_sb[:])
            nc.vector.scalar_tensor_tensor(out=vv[:], in0=ps_q[:], scalar=float(D),
                                           in1=sq[:], op0=ALU.mult,
                                           op1=ALU.subtract)
            # seed: y0 = bitcast(MAGIC+1 + ~(bits(vv) >> 1))
            y0i = y0.bitcast(I32)
            nc.vector.tensor_scalar(out=y0i[:], in0=vv.bitcast(I32)[:],
                                    scalar1=1, scalar2=-1,
                                    op0=ALU.logical_shift_right,
                                    op1=ALU.bitwise_xor)
            nc.vector.tensor_scalar(out=y0i[:], in0=y0i[:],
                                    scalar1=RSQRT_MAGIC + 1, scalar2=None,
                                    op0=ALU.add)
            # one NR pass: y1 = y0*(1.5 - 0.5*vv*y0^2)
            nc.vector.tensor_mul(t1[:], y0[:], y0[:])
            nc.vector.tensor_mul(t1[:], t1[:], vv[:])
            nc.vector.tensor_scalar(out=t1[:], in0=t1[:], scalar1=-0.5,
                                    scalar2=1.5, op0=ALU.mult, op1=ALU.add)
            nc.vector.tensor_mul(y0[:], t1[:], y0[:])       # y0 <- y1
            rb = small.tile([128, T], F32, name=f"rb_{lname}", tag="lnrb", bufs=1)
            mbrb = small.tile([128, T], F32, name=f"mb_{lname}", tag="lnmb", bufs=1)
            nc.vector.tensor_scalar_mul(rb[:], y0[:], float(D))
            nc.vector.tensor_mul(mbrb[:], s_sb[:], y0[:])
            if _dbg and lname == "l0a":
                nc.sync.dma_start(out=taps["tap_rb"].ap(), in_=rb[:])
                nc.sync.dma_start(out=taps["tap_mbrb"].ap(), in_=mbrb[:])
            h = actp.tile([128, FT, T], BF16, name=f"h_{lname}", tag="h")
            for ft in range(FT):
                z = scr.tile([128, T], F32, name=f"z_{lname}_{ft}", tag="scratch")
                nc.vector.tensor_mul(z[:], x_sb[:, ft, :], rb[:])
                nc.vector.tensor_sub(z[:], z[:], mbrb[:])
                nc.scalar.activation(out=h[:, ft, :], in_=z[:], func=AF.Identity,
                                     bias=sh_ap[:, ft:ft + 1],
                                     scale=sc_ap[:, ft:ft + 1])
            return h

        ones128 = singles.tile([128, 128], BF16, name="ones128")
        nc.vector.memset(ones128[:], 1.0)

        def rope_tile(dst, src_ap, lname):
            swp = scr.tile([128, T], BF16, name=f"swp_{lname}", tag="scr16")
            t1 = scr.tile([128, T], BF16, name=f"t1_{lname}", tag="scr16")
            for blk in range(4):
                sB = blk * 32
                oB = (blk ^ 1) * 32
                sgn = -1.0 if blk % 2 == 0 else 1.0
                nc.vector.tensor_scalar_mul(swp[sB:sB + 32, :],
                                            src_ap[oB:oB + 32, :], sgn)
            nc.vector.tensor_mul(swp[:], swp[:], cS[:])
            nc.vector.tensor_mul(t1[:], src_ap, cC[:])
            nc.vector.tensor_add(dst, t1[:], swp[:])

        for l in range(NL):
            park = parks[l]
            # ---- LN1 ----
            ps_s, ps_q = ln_stats(f"l{l}a")
            h = ln_finish(f"l{l}a", ps_s, ps_q, park[:, 1, :], park[:, 0, :])
            if _dbg and l == 0:
                nc.sync.dma_start(out=taps["tap_park"].ap(), in_=park[:])
                nc.sync.dma_start(out=taps["tap_h"].ap(), in_=h[:])

            # ---- qkv: k tiles first so the k AllGather launches early ----
            qk_sb = actp.tile([128, 16, T], BF16, name=f"qk_{l}", tag="qk")

            def qk_group(g):
                pss = group_psums(f"qk{l}_{g}")
                wt = wpool.tile([128, 4096], BF16, name="wqk_t", tag="w")
                nc.sync.dma_start(out=wt[:], in_=wqk[l, g])
                for k in range(KT):
                    for i in range(4):
                        nc.tensor.matmul(
                            pss[i],
                            lhsT=wt[:, k * 512 + i * 128:k * 512 + (i + 1) * 128],
                            rhs=h[:, k, :], start=(k == 0), stop=(k == KT - 1))
                for i in range(4):
                    ft = g * 4 + i
                    nc.scalar.activation(out=qk_sb[:, ft, :], in_=pss[i],
                                         func=AF.Identity,
                                         bias=bqk_sb[:, l, ft:ft + 1])

            for g in (2, 3):                      # k tiles 8..15
                qk_group(g)
                for i in range(4):
                    kt_ = (g - 2) * 4 + i
                    rope_tile(qk_sb[:, 8 + kt_, :], qk_sb[:, 8 + kt_, :],
                              f"rk{l}_{kt_}")
                    nc.gpsimd.dma_start(out=k_send[kt_], in_=qk_sb[:, 8 + kt_, :])
            nc.gpsimd.collective_compute(
                "AllGather", ALU.bypass,
                ins=[k_send.opt()], outs=[k_gath.opt()],
                replica_groups=PAIRS)

            # ---- v ----
            vloc = kvp.tile([128, 2, 4, 1024], BF16, name=f"vloc_{l}",
                            tag="vstage")
            for g in range(2):
                pss = group_psums(f"v{l}_{g}")
                wt = wpool.tile([128, 4096], BF16, name="wv_t", tag="w")
                nc.sync.dma_start(out=wt[:], in_=wv[l, g])
                for k in range(KT):
                    for i in range(4):
                        nc.tensor.matmul(
                            pss[i], lhsT=h[:, k, i * 128:(i + 1) * 128],
                            rhs=wt[:, k * 512:(k + 1) * 512],
                            start=(k == 0), stop=(k == KT - 1))
                for i in range(4):
                    nc.scalar.activation(out=vloc[:, 0, i, g * 512:(g + 1) * 512],
                                         in_=pss[i], func=AF.Copy)
            if use_vb:
                for i in range(4):
                    nc.vector.tensor_add(vloc[:, 0, i, :], vloc[:, 0, i, :],
                                         vb_sb[:, l, :])
            for i in range(4):
                nc.gpsimd.dma_start(out=v_send[i], in_=vloc[:, 0, i, :])
            nc.gpsimd.collective_compute(
                "AllGather", ALU.bypass,
                ins=[v_send.opt()], outs=[v_gath.opt()],
                replica_groups=PAIRS)

            # ---- q tiles + rope-q (overlap k/v AllGathers) ----
            for g in (0, 1):
                qk_group(g)
                for i in range(4):
                    qt_ = g * 4 + i
                    rope_tile(qk_sb[:, qt_, :], qk_sb[:, qt_, :], f"rq{l}_{qt_}")

            # ---- stage gathered k/v (emitted after the weight loads so the
            # sync queue never parks behind a collective semaphore) ----
            kfull = kvp.tile([128, 8, 1024], BF16, name=f"kfull_{l}", tag="kfull")
            for half in range(2):
                for pr in range(8):
                    nc.sync.dma_start(
                        out=kfull[:, pr, half * 512:(half + 1) * 512],
                        in_=k_gath[half, pr])
            vtmp = kvp.tile([128, 2, 4, 1024], BF16, name=f"vtmp_{l}",
                            tag="vstage")
            for half in range(2):
                for i in range(4):
                    nc.sync.dma_start(out=vtmp[:, half, i, :],
                                      in_=v_gath[half, i])
            for half in range(2):
                for i in range(4):
                    nc.vector.tensor_copy(
                        out=v_sb[:, half * 4 + i, :, 0:64],
                        in_=vtmp[:, half, i, :]
                        .rearrange("p (h d) -> p h d", h=H))

            if _dbg and l == 0:
                nc.sync.dma_start(out=taps["tap_qk"].ap(), in_=qk_sb[:])
                nc.sync.dma_start(out=taps["tap_kfull"].ap(), in_=kfull[:])
                nc.sync.dma_start(out=taps["tap_vsb"].ap(), in_=v_sb[:])

            # ---- attention: per-j interleaved scores -> exp -> out ----
            o_sb = actp.tile([128, FT, T], BF16, name=f"o_{l}", tag="o")
            for pr in range(8):
                acc_e = psA.tile([128, 512], F32, name=f"ae_{l}_{pr}", tag="acc")
                acc_o = psA.tile([128, 512], F32, name=f"ao_{l}_{pr}", tag="acc")
                pexps = [None] * 8

                def scores_j(j):
                    sc = psB.tile([128, 1024], F32, name=f"sc_{l}_{pr}_{j}",
                                  tag="sc2")
                    nc.tensor.matmul(sc[:, 0:512],
                                     lhsT=kfull[0:64, pr, j * 128:(j + 1) * 128],
                                     rhs=qk_sb[0:64, pr, :], start=True,
                                     stop=True)
                    nc.tensor.matmul(sc[:, 512:1024],
                                     lhsT=kfull[64:128, pr,
                                                j * 128:(j + 1) * 128],
                                     rhs=qk_sb[64:128, pr, :], start=True,
                                     stop=True)
                    pexp = ppool.tile([128, 1024], BF16,
                                      name=f"pexp_{l}_{pr}_{j}", tag="pexp",
                                      bufs=2)
                    nc.scalar.activation(out=pexp[:], in_=sc[:], func=AF.Exp,
                                         scale=SCALE)
                    pexps[j] = pexp

                def out_j(j):
                    nc.tensor.matmul(acc_e[:], lhsT=v_sb[:, j, 2 * pr, :],
                                     rhs=pexps[j][:, 0:512],
                                     start=(j == 0), stop=(j == 7))
                    nc.tensor.matmul(acc_o[:], lhsT=v_sb[:, j, 2 * pr + 1, :],
                                     rhs=pexps[j][:, 512:1024],
                                     start=(j == 0), stop=(j == 7))

                scores_j(0)
                for j in range(1, 8):
                    scores_j(j)
                    out_j(j - 1)
                out_j(7)

                # divide: denom replicated on partitions 64:128 via ones cols.
                # custom-DVE ops misread PSUM at partition offset 64 on real
                # hw (sim is fine) -- bounce the denom through SBUF first.
                for sub, acc in ((0, acc_e), (1, acc_o)):
                    den = small.tile([64, 512], F32, name=f"dn_{l}_{pr}_{sub}",
                                     tag="den", bufs=1)
                    nc.vector.tensor_copy(out=den[:], in_=acc[64:128, :])
                    rd = small.tile([64, 512], F32, name=f"rd_{l}_{pr}_{sub}",
                                    tag="rd")
                    nc.vector.reciprocal_approx_fast(out=rd[:], in_=den[:])
                    nc.vector.tensor_mul(o_sb[sub * 64:sub * 64 + 64, pr, :],
                                         acc[0:64, :], rd[:])

            # ---- proj + residual ----
            for g in range(2):
                pss = group_psums(f"pj{l}_{g}")
                wt = wpool.tile([128, 4096], BF16, name="wpj_t", tag="w")
                nc.sync.dma_start(out=wt[:], in_=wproj[l, g])
                for k in range(KT):
                    for i in range(4):
                        nc.tensor.matmul(
                            pss[i],
                            lhsT=wt[:, k * 512 + i * 128:k * 512 + (i + 1) * 128],
                            rhs=o_sb[:, k, :], start=(k == 0), stop=(k == KT - 1))
                for i in range(4):
                    ft = g * 4 + i
                    nc.vector.scalar_tensor_tensor(
                        out=x_sb[:, ft, :], in0=pss[i],
                        scalar=park[:, 2, ft:ft + 1], in1=x_sb[:, ft, :],
                        op0=ALU.mult, op1=ALU.add)
                    if use_pb:
                        gb = small.tile([128, 1], F32, name=f"gbp_{l}_{ft}",
                                        tag="gb")
                        nc.vector.tensor_mul(gb[:], park[:, 2, ft:ft + 1],
                                             bpj_sb[:, l, ft:ft + 1])
                        nc.vector.tensor_scalar_add(x_sb[:, ft, :],
                                                    x_sb[:, ft, :], gb[:])

            if _dbg and l == 0:
                nc.sync.dma_start(out=taps["tap_o"].ap(), in_=o_sb[:])
                nc.sync.dma_start(out=taps["tap_x1"].ap(), in_=x_sb[:])

            # ======== mlp ========
            ps_s2, ps_q2 = ln_stats(f"l{l}m")
            h2 = ln_finish(f"l{l}m", ps_s2, ps_q2, park[:, 4, :], park[:, 3, :])

            m1_sb = actp.tile([128, 32, T], BF16, name=f"m1_{l}", tag="m1")
            for g in range(8):
                pss = group_psums(f"m1{l}_{g}")
                wt = wpool.tile([128, 4096], BF16, name="wm1_t", tag="w")
                nc.sync.dma_start(out=wt[:], in_=wm1[l, g])
                for k in range(KT):
                    for i in range(4):
                        nc.tensor.matmul(
                            pss[i],
                            lhsT=wt[:, k * 512 + i * 128:k * 512 + (i + 1) * 128],
                            rhs=h2[:, k, :], start=(k == 0), stop=(k == KT - 1))
                for i in range(4):
                    mt = g * 4 + i
                    nc.scalar.activation(out=m1_sb[:, mt, :], in_=pss[i],
                                         func=AF.Gelu_apprx_tanh,
                                         bias=bm1_sb[:, l, mt:mt + 1])

            for g in range(2):
                pss = group_psums(f"m2{l}_{g}")
                for c in range(4):
                    wt = wpool.tile([128, 4096], BF16, name="wm2_t", tag="w")
                    nc.sync.dma_start(out=wt[:], in_=wm2[l, g * 4 + c])
                    for kk in range(KT):
                        k = c * 8 + kk
                        for i in range(4):
                            nc.tensor.matmul(
                                pss[i],
                                lhsT=wt[:, kk * 512 + i * 128:
                                        kk * 512 + (i + 1) * 128],
                                rhs=m1_sb[:, k, :], start=(k == 0),
                                stop=(k == 31))
                for i in range(4):
                    ft = g * 4 + i
                    nc.vector.scalar_tensor_tensor(
                        out=x_sb[:, ft, :], in0=pss[i],
                        scalar=park[:, 5, ft:ft + 1], in1=x_sb[:, ft, :],
                        op0=ALU.mult, op1=ALU.add)
                    if use_m2b:
                        gb = small.tile([128, 1], F32, name=f"gbm_{l}_{ft}",
                                        tag="gb")
                        nc.vector.tensor_mul(gb[:], park[:, 5, ft:ft + 1],
                                             bm2_sb[:, l, ft:ft + 1])
                        nc.vector.tensor_scalar_add(x_sb[:, ft, :],
                                                    x_sb[:, ft, :], gb[:])

        # final deferred scale + store
        alpha_fin = SKIP ** (2 * NL)
        for ft in range(FT):
            xo = scr.tile([128, T], F32, name=f"xo_{ft}", tag="scratch")
            nc.scalar.activation(out=xo[:], in_=x_sb[:, ft, :], func=AF.Copy,
                                 scale=alpha_fin)
            nc.sync.dma_start(out=out.ap()[:, ft, :], in_=xo[:])
    return nc


def _fat(w_groups):
    """[NL, G, KT, 128, 512] -> [NL, G, 128, KT*512] contiguous rows."""
    nl, g, kt, p, c = w_groups.shape
    return np.ascontiguousarray(
        w_groups.transpose(0, 1, 3, 2, 4).reshape(nl, g, p, kt * c))


def _pack_inputs(inputs):
    x = np.asarray(inputs["x"], np.float32)
    c = np.asarray(inputs["c"], np.float32)
    t = np.asarray(inputs["t"], np.float32)
    qkv_w = np.asarray(inputs["qkv_w"], np.float32)
    qkv_b = np.asarray(inputs["qkv_b"], np.float32)
    proj_w = np.asarray(inputs["proj_w"], np.float32)
    proj_b = np.asarray(inputs["proj_b"], np.float32)
    mlp_w1 = np.asarray(inputs["mlp_w1"], np.float32)
    mlp_b1 = np.asarray(inputs["mlp_b1"], np.float32)
    mlp_w2 = np.asarray(inputs["mlp_w2"], np.float32)
    mlp_b2 = np.asarray(inputs["mlp_b2"], np.float32)
    adaln_w = np.asarray(inputs["adaln_w"], np.float32)
    adaln_b = np.asarray(inputs["adaln_b"], np.float32)

    perm = _deinterleave_perm()
    wq = qkv_w[:, :, 0:D][:, :, perm]
    wk = qkv_w[:, :, D:2 * D][:, :, perm]
    wqk = np.concatenate([wq, wk], axis=2)                       # [NL, D, 2D]
    wqk_pack = _fat(
        wqk.reshape(NL, KT, 128, 4, 512).transpose(0, 3, 1, 2, 4)).astype(BF)
    wv_pack = _fat(
        qkv_w[:, :, 2 * D:].reshape(NL, KT, 128, 2, 512)
        .transpose(0, 3, 1, 2, 4)).astype(BF)
    wpj_pack = _fat(
        proj_w.reshape(NL, KT, 128, 2, 512).transpose(0, 3, 1, 2, 4)).astype(BF)
    wm1_pack = _fat(
        mlp_w1.reshape(NL, KT, 128, 8, 512).transpose(0, 3, 1, 2, 4)).astype(BF)
    # wm2: [NL, 4D, D] -> groups g(2) x chunks c4(4) of 8 k-tiles each
    a = mlp_w2.reshape(NL, 32, 128, 2, 512).transpose(0, 3, 1, 2, 4)
    a = a.reshape(NL, 2, 4, 8, 128, 512).transpose(0, 1, 2, 4, 3, 5)
    wm2_pack = np.ascontiguousarray(a.reshape(NL, 8, 128, KT * 512)).astype(BF)

    bqk_v = np.concatenate([qkv_b[:, 0:D][:, perm],
                            qkv_b[:, D:2 * D][:, perm]], 1)
    bqk_pack = np.ascontiguousarray(
        bqk_v.reshape(NL, 16, 128).transpose(0, 2, 1)).astype(np.float32)
    bm1_pack = np.ascontiguousarray(
        mlp_b1.reshape(NL, 32, 128).transpose(0, 2, 1)).astype(np.float32)
    vb = qkv_b[:, 2 * D:]
    use_vb = bool(np.any(vb != 0))
    use_pb = bool(np.any(proj_b != 0))
    use_m2b = bool(np.any(mlp_b2 != 0))
    use_adb = bool(np.any(adaln_b != 0))

    pos = np.arange(L, dtype=np.float32)
    omega = 1.0 / (10000.0 ** (np.arange(0, HD, 2, dtype=np.float32) / HD))
    ang = pos[:, None] * omega[None, :]
    cosT = np.cos(ang).T.astype(np.float32)                      # [32, L]
    sinT = np.sin(ang).T.astype(np.float32)

    cc = (c[:, 0, :] + t) * SKIP                                 # [B, D]
    silu_cc = (cc / (1.0 + np.exp(-cc))).astype(np.float32)
    scc_pack = np.ascontiguousarray(
        silu_cc.T.reshape(KT, 128, B).transpose(1, 0, 2)).astype(BF)

    per_core = []
    for cid in range(NC):
        b, half = cid // 2, cid % 2
        l0 = half * T
        xt = x[b, l0:l0 + T, :].T                                # [D, T]
        xt_pack = np.ascontiguousarray(
            xt.reshape(FT, 128, T).transpose(1, 0, 2)).astype(np.float32)
        # wad fat: [NL, 2, 128, 4*ADC], chunk c holds k-tiles 4c..4c+3
        adw = adaln_w[:, :, cid * ADC:(cid + 1) * ADC].reshape(NL, 2, 4, 128, ADC)
        wad_pack = np.ascontiguousarray(
            adw.transpose(0, 1, 3, 2, 4).reshape(NL, 2, 128, 4 * ADC)).astype(BF)
        m = {
            "xt": xt_pack,
            "silu_cc": scc_pack,
            "ropeC": np.ascontiguousarray(
                np.tile(cosT[:, l0:l0 + T], (4, 1))).astype(BF),
            "ropeS": np.ascontiguousarray(
                np.tile(sinT[:, l0:l0 + T], (4, 1))).astype(BF),
            "wqk": wqk_pack, "wv": wv_pack, "wproj": wpj_pack,
            "wm1": wm1_pack, "wm2": wm2_pack,
            "wad": wad_pack,
            "bqk": bqk_pack, "bm1": bm1_pack,
        }
        if use_vb:
            m["vb_b"] = np.ascontiguousarray(
                np.broadcast_to(vb[:, None, :], (NL, 128, 1024))).astype(np.float32)
        if use_pb:
            m["bpj"] = np.ascontiguousarray(
                proj_b.reshape(NL, FT, 128).transpose(0, 2, 1)).astype(np.float32)
        if use_m2b:
            m["bm2"] = np.ascontiguousarray(
                mlp_b2.reshape(NL, FT, 128).transpose(0, 2, 1)).astype(np.float32)
        if use_adb:
            m["bad"] = np.ascontiguousarray(
                adaln_b.reshape(NL, 48, 128).transpose(0, 2, 1)).astype(np.float32)
        per_core.append(m)
    return per_core, (use_vb, use_pb, use_m2b, use_adb)


_CACHE = {}


def _get_nc(flags):
    if flags not in _CACHE:
        nc = bacc.Bacc("TRN2", target_bir_lowering=False, debug=False,
                       num_devices=NC)
        build(nc, *flags)
        nc.compile()
        _CACHE[flags] = nc
    return _CACHE[flags]


def kernel(**inputs) -> np.ndarray:
    in_maps, flags = _pack_inputs(inputs)
    nc = _get_nc(flags)
    res = run_bass_kernel_spmd(nc, in_maps, core_ids=list(range(NC)))
    full = np.zeros((B, L, D), np.float32)
    for cid in range(NC):
        b, half = cid // 2, cid % 2
        l0 = half * T
        o = np.asarray(res.results[cid]["out"])                  # [128, FT, T]
        full[b, l0:l0 + T, :] = o.transpose(1, 0, 2).reshape(D, T).T
    return full
